# revision 1
# baseline (speedup 1.0000x reference)
"""ACDA (adaptive conv) Trainium2 kernel — 8-core data-parallel over batch.

Per core: one sample (C=64, H=128, W=128). The image is split into two
64-row halves stacked on the 128 SBUF partitions (partition p<64 -> half A
channel p, p>=64 -> half B channel p-64), so every engine op runs with all
128 lanes active; the two halves' matmuls run concurrently in opposite
quadrants of the PE array (tile_position (0,0) / (64,64)).

Host-side prep (inside kernel(), not on device): x is zero-padded, bf16-cast
and laid out per-core as two copies (xpadA with a left pad column, xpadB
column-shifted by one) so that all nine 3x3-tap shifts are 4-byte-aligned
SBUF views; weights are pre-transposed to lhsT layout (c_in, k, c_out).

Pipeline per 16-row tile (per half), fully overlapped by the Tile scheduler:
  DMA: padded x arrives in 4 row-bands (1 tile + halo each)
  PE:  g_k = W_k^T @ x  (bf16 in, fp32 PSUM), per kernel-position k (9x),
       4 chunks x 2 halves per k, one PSUM bank per chunk
  ACT: f_k = relu(g_k + b_k)  PSUM -> SBUF bf16  (bias is per-partition)
  DVE: prod = f * patch, one op per dj (di rides the access pattern)
  DVE: tree levels 1-2 in place inside prod (prod[4:8] += prod[0:4]; ...)
  GP:  tree tail (prod[7]+=prod[6]; prod[8]+=prod[7]) on GPSIMD to unload DVE
  DMA: prod[:,8] tile -> DRAM in bf16 (widened to fp32 on the host)
"""

import numpy as np
import ml_dtypes
from contextlib import ExitStack

import concourse.bass as bass
import concourse.tile as tile
from concourse import bacc, mybir
from concourse.bass_utils import run_bass_kernel_spmd

B, C, H, W, K = 8, 64, 128, 128, 3
NCORES = 8
RT = 16           # output rows per tile (per half)
BF16 = mybir.dt.bfloat16
F32 = mybir.dt.float32
RELU = mybir.ActivationFunctionType.Relu
MULT = mybir.AluOpType.mult
ADD = mybir.AluOpType.add

_CACHE = {}


def _window3(ap, lr0, dj, rt):
    """Overlapping 3-row window AP: [128, 3(di), rt, 128] over a padded-x
    band tile [128, nrows, rowlen], starting at buffer row lr0, col dj."""
    a = ap.copy()
    v = a.ap
    row_stride = v[1][0]
    v[1] = [row_stride, 3]
    v[2] = [row_stride, rt]
    v.append([1, 128])
    a.offset = a.offset + lr0 * row_stride + dj
    return a


def _kernel_body(ctx: ExitStack, tc, out_d, xA_d, xB_d, wT_d, bias_d, nreps=1,
                 opts=None):
    nc = tc.nc
    o = dict(rt=RT, psum_bufs=2, fbufs=2, pbufs=2, gp=2, m3=False)
    o.update(opts or {})
    rt = o["rt"]
    nt = 64 // rt
    nch = rt // 4          # 4-row matmul chunks per k-tile (1 PSUM bank each)

    inp = ctx.enter_context(tc.tile_pool(name="inp", bufs=1))
    # dep-free dummy relu so the one-time ACT table load runs at t~0 instead
    # of gating the first real eviction
    scratch = inp.tile([128, 2], F32)
    nc.gpsimd.memset(scratch[:], 0.0)
    nc.scalar.activation(scratch[:, 0:1], scratch[:, 1:2], RELU, bias=0.0)
    # dep-free dummy matmuls so the PE p-state ramps to full clock while the
    # input DMA is still in flight (HAM needs ~3us of sustained PE activity)
    warm = inp.tile([128, 512], BF16)
    nc.vector.memset(warm[:], 0.0)
    wT = inp.tile([128, 9, 64], BF16)            # lhsT per k, duplicated on halves
    bias = inp.tile([128, 9], F32)
    # x staged in 4 row-bands (16 rows + halo each) so the first matmuls can
    # start after ~1/4 of the input DMA. Band 0 is issued before the weights:
    # its transfer is the long pole for the first eviction.
    xAb = [inp.tile([128, 18, 130], BF16, name=f"xAb{b}", tag=f"xA{b}")
           for b in range(4)]
    xBb = [inp.tile([128, 18, 128], BF16, name=f"xBb{b}", tag=f"xB{b}")
           for b in range(4)]
    nc.sync.dma_start(wT[:], wT_d[:])
    nc.sync.dma_start(xAb[0][:], xA_d[:, 0:18, :])
    nc.sync.dma_start(bias[:], bias_d[:])
    nc.sync.dma_start(xBb[0][:], xB_d[:, 0:18, :])
    for b in range(1, 4):
        nc.sync.dma_start(xAb[b][:], xA_d[:, 16 * b: 16 * b + 18, :])
        nc.sync.dma_start(xBb[b][:], xB_d[:, 16 * b: 16 * b + 18, :])

    psum = ctx.enter_context(tc.tile_pool(name="psum", bufs=o["psum_bufs"],
                                          space="PSUM"))
    fpool = ctx.enter_context(tc.tile_pool(name="f", bufs=o["fbufs"]))
    ppool = ctx.enter_context(tc.tile_pool(name="prod", bufs=o["pbufs"]))

    if o.get("warm", 8):
        wps = psum.tile([128, rt, 128], F32, name="warm_ps", tag="ps")
        for i in range(o.get("warm", 8)):
            nc.tensor.matmul(wps[0:64, 0:4, :], warm[0:64, 0:64],
                             warm[0:64].rearrange("p (a b) -> p a b", a=4),
                             start=True, stop=True)

    out4 = out_d.rearrange("c (h r) w -> h c r w", h=2)

    for t in range(nt * nreps):
        r0 = (t % nt) * rt
        band = r0 // 16
        lr0 = r0 - 16 * band
        xA, xB = xAb[band], xBb[band]
        f = fpool.tile([128, 9, rt, 128], BF16)
        prod = ppool.tile([128, 9, rt, 128], BF16)

        # The whole post-eviction pipeline (muls + in-place reduction tree)
        # is row-split between DVE (rows 0:s) and GPSIMD (rows s:rt): two
        # independent streams that never wait on each other, sized so both
        # fit under the ACT eviction pace. In-place tree: out AP == operand
        # AP is safe (write pointer trails read pointer). Muls and adds are
        # interleaved in emission order so the strict engine FIFOs always
        # have ready work at the head; the last-evicted tap k=8 joins the
        # root in the single final add, so the tile drain is mul8+add+DMA.
        s = o.get("split", 13) if o.get("gp", 1) else rt
        engs = [(nc.vector, 0, s)]
        if s < rt:
            engs.append((nc.gpsimd, s, rt))

        def kgroup(k):
            ps = psum.tile([128, rt, 128], F32)
            if o.get("stage") != "nomm":
                for h in (0, 1):
                    p0 = 64 * h
                    # center pixels: buffer row r+1, buffer cols 1..128
                    for n in range(nch):  # <=512 fp32 cols per matmul (1 bank)
                        rhs = xA[p0:p0 + 64, lr0 + 1 + 4 * n: lr0 + 5 + 4 * n, 1:129]
                        nc.tensor.matmul(
                            ps[p0:p0 + 64, 4 * n: 4 * n + 4, :],
                            wT[p0:p0 + 64, k, :],
                            rhs,
                            start=True, stop=True,
                        )
            if o.get("stage") != "noact":
                nc.scalar.activation(f[:, k], ps[:], RELU, bias=bias[:, k:k + 1])

        def mul(k):
            di, dj = divmod(k, 3)
            for e, r_a, r_b in engs:
                if dj == 1:
                    patch = xB[:, lr0 + di + r_a: lr0 + di + r_b, 0:128]
                else:
                    patch = xA[:, lr0 + di + r_a: lr0 + di + r_b, dj:dj + 128]
                e.tensor_tensor(prod[:, k, r_a:r_b], f[:, k, r_a:r_b], patch,
                                op=MULT)

        def add(dst, src):
            for e, r_a, r_b in engs:
                e.tensor_tensor(prod[:, dst, r_a:r_b], prod[:, src, r_a:r_b],
                                prod[:, dst, r_a:r_b], op=ADD)

        stage = o.get("stage", "full")
        if stage == "nomm":
            # pure ACT pace probe: all evictions read one pre-filled psum tile
            if t == 0:
                pse = psum.tile([128, rt, 128], F32, tag="ps")
                for n in range(nch):
                    nc.tensor.matmul(pse[0:64, 4 * n: 4 * n + 4, :],
                                     wT[0:64, 0, :],
                                     xA[0:64, 1 + 4 * n: 5 + 4 * n, 1:129],
                                     start=True, stop=True)
                o["_pse"] = pse
            for k in range(9):
                nc.scalar.activation(f[:, k], o["_pse"][:], RELU,
                                     bias=bias[:, k:k + 1])
            nc.sync.dma_start(out4[:, :, r0:r0 + rt, :], f[:, 8])
            continue
        if stage == "noact":
            # pure PE pace probe: matmuls only, DMA from the input band
            for k in range(9):
                kgroup(k)
            nc.sync.dma_start(out_d[:, r0:r0 + rt, :],
                              xA[0:64, 1:1 + rt, 1:129])
            continue
        if stage == "empty":
            # loop-overhead floor: input DMAs + output DMAs only
            nc.sync.dma_start(out_d[:, r0:r0 + rt, :],
                              xA[0:64, 1:1 + rt, 1:129])
            continue
        if stage == "evict":
            for k in range(9):
                kgroup(k)
            nc.sync.dma_start(out4[:, :, r0:r0 + rt, :], f[:, 8])
            continue
        if stage == "noadd":
            for k in range(9):
                kgroup(k)
            for k in range(9):
                mul(k)
            nc.sync.dma_start(out4[:, :, r0:r0 + rt, :], prod[:, 8])
            continue

        for k in range(8):
            kgroup(k)
        mul(0)
        mul(1)
        add(1, 0)
        mul(2)
        mul(3)
        add(3, 2)
        add(3, 1)
        mul(4)
        mul(5)
        add(5, 4)
        mul(6)
        mul(7)
        add(7, 6)
        add(7, 5)
        add(7, 3)

        kgroup(8)
        mul(8)
        add(8, 7)

        for i in range(o.get("gpx", 0)):
            # marginal-GPSIMD-cost probe: dead big-FD TT adds on Pool
            nc.gpsimd.tensor_tensor(prod[:, (i % 7)], prod[:, (i % 7)],
                                    prod[:, (i % 7) + 1], op=ADD)

        nc.sync.dma_start(out4[:, :, r0:r0 + rt, :], prod[:, 8])


def _build():
    if "nc" in _CACHE:
        return _CACHE["nc"]
    nc = bacc.Bacc("TRN2", target_bir_lowering=False, debug=False,
                   num_devices=NCORES)
    xA_d = nc.dram_tensor("xpadA", (128, 66, 130), BF16, kind="ExternalInput").ap()
    xB_d = nc.dram_tensor("xpadB", (128, 66, 128), BF16, kind="ExternalInput").ap()
    wT_d = nc.dram_tensor("wT", (128, 9, 64), BF16, kind="ExternalInput").ap()
    bias_d = nc.dram_tensor("bias", (128, 9), F32, kind="ExternalInput").ap()
    out_d = nc.dram_tensor("out", (C, H, W), BF16, kind="ExternalOutput").ap()
    with tile.TileContext(nc) as tc, ExitStack() as ctx:
        _kernel_body(ctx, tc, out_d, xA_d, xB_d, wT_d, bias_d)
    nc.compile()
    _CACHE["nc"] = nc
    return nc


def _prep_core_inputs(x_i: np.ndarray, wT_np, bias_np):
    """x_i: (C, H, W) float32 -> per-core input dict."""
    bf = ml_dtypes.bfloat16
    xA = np.zeros((128, 66, 130), dtype=bf)
    xB = np.zeros((128, 66, 128), dtype=bf)
    xb = x_i.astype(bf)
    # half A: buffer rows 0..65 = x rows -1..64 (row -1 zero-padded)
    xA[0:64, 1:66, 1:129] = xb[:, 0:65, :]
    xB[0:64, 1:66, :] = xb[:, 0:65, :]
    # half B: buffer rows 0..65 = x rows 63..128 (row 128 zero-padded)
    xA[64:128, 0:65, 1:129] = xb[:, 63:128, :]
    xB[64:128, 0:65, :] = xb[:, 63:128, :]
    return {"xpadA": xA, "xpadB": xB, "wT": wT_np, "bias": bias_np}


def kernel(x: np.ndarray, W_gen: np.ndarray, b_gen: np.ndarray) -> np.ndarray:
    x = np.asarray(x, dtype=np.float32)
    W_gen = np.asarray(W_gen, dtype=np.float32)
    b_gen = np.asarray(b_gen, dtype=np.float32)

    nc = _build()

    bf = ml_dtypes.bfloat16
    # lhsT: (c_in, k, c_out); o index in reference = c_out*9 + k
    wT_half = W_gen.reshape(C, K * K, C).transpose(2, 1, 0).astype(bf)  # (cin,k,cout)
    wT_np = np.ascontiguousarray(np.concatenate([wT_half, wT_half], axis=0))
    b2 = b_gen.reshape(C, K * K).astype(np.float32)                     # (c_out, k)
    bias_np = np.ascontiguousarray(np.concatenate([b2, b2], axis=0))    # (128, 9)

    in_maps = [_prep_core_inputs(x[i], wT_np, bias_np) for i in range(NCORES)]
    res = run_bass_kernel_spmd(nc, in_maps, core_ids=list(range(NCORES)))
    out = np.stack([res.results[i]["out"] for i in range(NCORES)], axis=0)
    return out.astype(np.float32)


if __name__ == "__main__":
    xs = np.random.randn(B, C, H, W).astype(np.float32)
    Wg = np.random.randn(C * K * K, C).astype(np.float32) / np.sqrt(C)
    bg = (np.random.randn(C * K * K) * 0.01).astype(np.float32)
    o = kernel(xs, Wg, bg)
    print("out", o.shape, o.dtype, float(np.abs(o).mean()))



# revision 7
# speedup vs baseline: 1.0550x; 1.0550x over previous
"""ACDA (adaptive conv) Trainium2 kernel — 8-core data-parallel over batch.

Per core: one sample (C=64, H=128, W=128). The image is split into two
64-row halves stacked on the 128 SBUF partitions (partition p<64 -> half A
channel p, p>=64 -> half B channel p-64), so every engine op runs with all
128 lanes active; the two halves' matmuls run concurrently in opposite
quadrants of the PE array (tile_position (0,0) / (64,64)).

Host-side prep (inside kernel(), not on device): x is zero-padded, bf16-cast
and laid out per-core as two copies (xpadA with a left pad column, xpadB
column-shifted by one) so that all nine 3x3-tap shifts are 4-byte-aligned
SBUF views; weights are pre-transposed to lhsT layout (c_in, k, c_out).

Pipeline per 16-row tile (per half), fully overlapped by the Tile scheduler:
  DMA: padded x arrives in 4 row-bands (1 tile + halo each)
  PE:  g_k = W_k^T @ x  (bf16 in, fp32 PSUM), per kernel-position k (9x),
       4 chunks x 2 halves per k, one PSUM bank per chunk
  ACT: f_k = relu(g_k + b_k)  PSUM -> SBUF bf16  (bias is per-partition)
  DVE: prod = f * patch, one op per dj (di rides the access pattern)
  DVE: tree levels 1-2 in place inside prod (prod[4:8] += prod[0:4]; ...)
  GP:  tree tail (prod[7]+=prod[6]; prod[8]+=prod[7]) on GPSIMD to unload DVE
  DMA: prod[:,8] tile -> DRAM in bf16 (widened to fp32 on the host)
"""

import numpy as np
import ml_dtypes
from contextlib import ExitStack

import concourse.bass as bass
import concourse.tile as tile
from concourse import bacc, mybir
from concourse.bass_utils import run_bass_kernel_spmd

B, C, H, W, K = 8, 64, 128, 128, 3
NCORES = 8
RT = 16           # output rows per tile (per half)
BF16 = mybir.dt.bfloat16
F32 = mybir.dt.float32
RELU = mybir.ActivationFunctionType.Relu
MULT = mybir.AluOpType.mult
ADD = mybir.AluOpType.add

_CACHE = {}


def _window3(ap, lr0, dj, rt):
    """Overlapping 3-row window AP: [128, 3(di), rt, 128] over a padded-x
    band tile [128, nrows, rowlen], starting at buffer row lr0, col dj."""
    a = ap.copy()
    v = a.ap
    row_stride = v[1][0]
    v[1] = [row_stride, 3]
    v[2] = [row_stride, rt]
    v.append([1, 128])
    a.offset = a.offset + lr0 * row_stride + dj
    return a


def _kernel_body(ctx: ExitStack, tc, out_d, xA_d, xB_d, wT_d, bias_d, nreps=1,
                 opts=None):
    nc = tc.nc
    o = dict(rt=RT, psum_bufs=2, fbufs=2, pbufs=2, gp=2, m3=False)
    o.update(opts or {})
    rt = o["rt"]
    nt = 64 // rt
    nch = rt // 4          # 4-row matmul chunks per k-tile (1 PSUM bank each)

    inp = ctx.enter_context(tc.tile_pool(name="inp", bufs=1))
    # dep-free dummy relu so the one-time ACT table load runs at t~0 instead
    # of gating the first real eviction
    scratch = inp.tile([128, 2], F32)
    nc.gpsimd.memset(scratch[:], 0.0)
    nc.scalar.activation(scratch[:, 0:1], scratch[:, 1:2], RELU, bias=0.0)
    # dep-free dummy matmuls so the PE p-state ramps to full clock while the
    # input DMA is still in flight (HAM needs ~3us of sustained PE activity)
    warm = inp.tile([128, 512], BF16)
    nc.vector.memset(warm[:], 0.0)
    # lhsT per k: block-diagonal [[W_k, 0], [0, W_k]] so one 128-contraction
    # matmul computes tap k for BOTH image halves in a single rhs stream
    # (the 64-deep quadrant split would stream every pixel chunk twice)
    wT = inp.tile([128, 9, 128], BF16)
    bias = inp.tile([128, 9], F32)
    # x staged in 4 row-bands (16 rows + halo each) so the first matmuls can
    # start after ~1/4 of the input DMA. Band 0 is issued before the weights:
    # its transfer is the long pole for the first eviction.
    xAb = [inp.tile([128, 18, 130], BF16, name=f"xAb{b}", tag=f"xA{b}")
           for b in range(4)]
    xBb = [inp.tile([128, 18, 128], BF16, name=f"xBb{b}", tag=f"xB{b}")
           for b in range(4)]
    nc.sync.dma_start(wT[:], wT_d[:])
    nc.sync.dma_start(xAb[0][:], xA_d[:, 0:18, :])
    nc.sync.dma_start(bias[:], bias_d[:])
    nc.sync.dma_start(xBb[0][:], xB_d[:, 0:18, :])
    for b in range(1, 4):
        nc.sync.dma_start(xAb[b][:], xA_d[:, 16 * b: 16 * b + 18, :])
        nc.sync.dma_start(xBb[b][:], xB_d[:, 16 * b: 16 * b + 18, :])

    psum = ctx.enter_context(tc.tile_pool(name="psum", bufs=o["psum_bufs"],
                                          space="PSUM"))
    fpool = ctx.enter_context(tc.tile_pool(name="f", bufs=o["fbufs"]))
    ppool = ctx.enter_context(tc.tile_pool(name="prod", bufs=o["pbufs"]))

    if o.get("warm", 8):
        wps = psum.tile([128, rt, 128], F32, name="warm_ps", tag="ps")
        for i in range(o.get("warm", 8)):
            nc.tensor.matmul(wps[0:64, 0:4, :], warm[0:64, 0:64],
                             warm[0:64].rearrange("p (a b) -> p a b", a=4),
                             start=True, stop=True)

    out4 = out_d.rearrange("c (h r) w -> h c r w", h=2)

    for t in range(nt * nreps):
        r0 = (t % nt) * rt
        band = r0 // 16
        lr0 = r0 - 16 * band
        xA, xB = xAb[band], xBb[band]
        f = fpool.tile([128, 9, rt, 128], BF16)
        prod = ppool.tile([128, 9, rt, 128], BF16)

        # The whole post-eviction pipeline (muls + in-place reduction tree)
        # is row-split between DVE (rows 0:s) and GPSIMD (rows s:rt): two
        # independent streams that never wait on each other, sized so both
        # fit under the ACT eviction pace. In-place tree: out AP == operand
        # AP is safe (write pointer trails read pointer). Muls and adds are
        # interleaved in emission order so the strict engine FIFOs always
        # have ready work at the head; the last-evicted tap k=8 joins the
        # root in the single final add, so the tile drain is mul8+add+DMA.
        s = o.get("split", 13) if o.get("gp", 1) else rt
        engs = [(nc.vector, 0, s)]
        if s < rt:
            engs.append((nc.gpsimd, s, rt))

        def kgroup(k):
            ps = psum.tile([128, rt, 128], F32)
            if o.get("stage") != "nomm":
                # center pixels: buffer row r+1, buffer cols 1..128
                for n in range(nch):  # <=512 fp32 cols per matmul (1 bank)
                    rhs = xA[:, lr0 + 1 + 4 * n: lr0 + 5 + 4 * n, 1:129]
                    nc.tensor.matmul(
                        ps[:, 4 * n: 4 * n + 4, :],
                        wT[:, k, :],
                        rhs,
                        start=True, stop=True,
                    )
            if o.get("stage") != "noact":
                nc.scalar.activation(f[:, k], ps[:], RELU, bias=bias[:, k:k + 1])

        def mul(k):
            di, dj = divmod(k, 3)
            for e, r_a, r_b in engs:
                if dj == 1:
                    patch = xB[:, lr0 + di + r_a: lr0 + di + r_b, 0:128]
                else:
                    patch = xA[:, lr0 + di + r_a: lr0 + di + r_b, dj:dj + 128]
                e.tensor_tensor(prod[:, k, r_a:r_b], f[:, k, r_a:r_b], patch,
                                op=MULT)

        def add(dst, src):
            for e, r_a, r_b in engs:
                e.tensor_tensor(prod[:, dst, r_a:r_b], prod[:, src, r_a:r_b],
                                prod[:, dst, r_a:r_b], op=ADD)

        stage = o.get("stage", "full")
        if stage == "nomm":
            # pure ACT pace probe: all evictions read one pre-filled psum tile
            if t == 0:
                pse = psum.tile([128, rt, 128], F32, tag="ps")
                for n in range(nch):
                    nc.tensor.matmul(pse[:, 4 * n: 4 * n + 4, :],
                                     wT[:, 0, :],
                                     xA[:, 1 + 4 * n: 5 + 4 * n, 1:129],
                                     start=True, stop=True)
                o["_pse"] = pse
            for k in range(9):
                nc.scalar.activation(f[:, k], o["_pse"][:], RELU,
                                     bias=bias[:, k:k + 1])
            nc.sync.dma_start(out4[:, :, r0:r0 + rt, :], f[:, 8])
            continue
        if stage == "noact":
            # pure PE pace probe: matmuls only, DMA from the input band
            for k in range(9):
                kgroup(k)
            nc.sync.dma_start(out_d[:, r0:r0 + rt, :],
                              xA[0:64, 1:1 + rt, 1:129])
            continue
        if stage == "empty":
            # loop-overhead floor: input DMAs + output DMAs only
            nc.sync.dma_start(out_d[:, r0:r0 + rt, :],
                              xA[0:64, 1:1 + rt, 1:129])
            continue
        if stage == "evict":
            for k in range(9):
                kgroup(k)
            nc.sync.dma_start(out4[:, :, r0:r0 + rt, :], f[:, 8])
            continue
        if stage == "noadd":
            for k in range(9):
                kgroup(k)
            for k in range(9):
                mul(k)
            nc.sync.dma_start(out4[:, :, r0:r0 + rt, :], prod[:, 8])
            continue

        for k in range(8):
            kgroup(k)
        mul(0)
        mul(1)
        add(1, 0)
        mul(2)
        mul(3)
        add(3, 2)
        add(3, 1)
        mul(4)
        mul(5)
        add(5, 4)
        mul(6)
        mul(7)
        add(7, 6)
        add(7, 5)
        add(7, 3)

        kgroup(8)
        mul(8)
        add(8, 7)

        for i in range(o.get("gpx", 0)):
            # marginal-GPSIMD-cost probe: dead big-FD TT adds on Pool
            nc.gpsimd.tensor_tensor(prod[:, (i % 7)], prod[:, (i % 7)],
                                    prod[:, (i % 7) + 1], op=ADD)

        nc.sync.dma_start(out4[:, :, r0:r0 + rt, :], prod[:, 8])


def _declare_tensors(nc):
    xA_d = nc.dram_tensor("xpadA", (128, 66, 130), BF16, kind="ExternalInput").ap()
    xB_d = nc.dram_tensor("xpadB", (128, 66, 128), BF16, kind="ExternalInput").ap()
    wT_d = nc.dram_tensor("wT", (128, 9, 128), BF16, kind="ExternalInput").ap()
    bias_d = nc.dram_tensor("bias", (128, 9), F32, kind="ExternalInput").ap()
    out_d = nc.dram_tensor("out", (C, H, W), BF16, kind="ExternalOutput").ap()
    return out_d, xA_d, xB_d, wT_d, bias_d


def _build():
    if "nc" in _CACHE:
        return _CACHE["nc"]
    nc = bacc.Bacc("TRN2", target_bir_lowering=False, debug=False,
                   num_devices=NCORES)
    aps = _declare_tensors(nc)
    with tile.TileContext(nc) as tc, ExitStack() as ctx:
        _kernel_body(ctx, tc, *aps)
    nc.compile()
    _CACHE["nc"] = nc
    return nc


def _prep_core_inputs(x_i: np.ndarray, wT_np, bias_np):
    """x_i: (C, H, W) float32 -> per-core input dict."""
    bf = ml_dtypes.bfloat16
    xA = np.zeros((128, 66, 130), dtype=bf)
    xB = np.zeros((128, 66, 128), dtype=bf)
    xb = x_i.astype(bf)
    # half A: buffer rows 0..65 = x rows -1..64 (row -1 zero-padded)
    xA[0:64, 1:66, 1:129] = xb[:, 0:65, :]
    xB[0:64, 1:66, :] = xb[:, 0:65, :]
    # half B: buffer rows 0..65 = x rows 63..128 (row 128 zero-padded)
    xA[64:128, 0:65, 1:129] = xb[:, 63:128, :]
    xB[64:128, 0:65, :] = xb[:, 63:128, :]
    return {"xpadA": xA, "xpadB": xB, "wT": wT_np, "bias": bias_np}


def _prep_inputs(x, W_gen, b_gen):
    x = np.asarray(x, dtype=np.float32)
    W_gen = np.asarray(W_gen, dtype=np.float32)
    b_gen = np.asarray(b_gen, dtype=np.float32)

    bf = ml_dtypes.bfloat16
    # lhsT: (c_in, k, c_out); o index in reference = c_out*9 + k.
    # Block-diagonal on (cin, cout) so one matmul serves both image halves.
    wT_half = W_gen.reshape(C, K * K, C).transpose(2, 1, 0).astype(bf)  # (cin,k,cout)
    wT_np = np.zeros((128, K * K, 128), dtype=bf)
    wT_np[0:C, :, 0:C] = wT_half
    wT_np[C:128, :, C:128] = wT_half
    b2 = b_gen.reshape(C, K * K).astype(np.float32)                     # (c_out, k)
    bias_np = np.ascontiguousarray(np.concatenate([b2, b2], axis=0))    # (128, 9)

    return [_prep_core_inputs(x[i], wT_np, bias_np) for i in range(x.shape[0])]


def kernel(x: np.ndarray, W_gen: np.ndarray, b_gen: np.ndarray) -> np.ndarray:
    nc = _build()
    in_maps = _prep_inputs(x, W_gen, b_gen)
    res = run_bass_kernel_spmd(nc, in_maps, core_ids=list(range(NCORES)))
    out = np.stack([res.results[i]["out"] for i in range(NCORES)], axis=0)
    return out.astype(np.float32)


if __name__ == "__main__":
    xs = np.random.randn(B, C, H, W).astype(np.float32)
    Wg = np.random.randn(C * K * K, C).astype(np.float32) / np.sqrt(C)
    bg = (np.random.randn(C * K * K) * 0.01).astype(np.float32)
    o = kernel(xs, Wg, bg)
    print("out", o.shape, o.dtype, float(np.abs(o).mean()))

